# revision 1
# baseline (speedup 1.0000x reference)
"""GraphSAGE 2-layer forward on 8 TRN2 NeuronCores.

Strategy (graph/data parallel per sharding hint):
- Nodes dst-sharded across 8 cores (6250 nodes/core, 49 tiles of 128).
- Host sorts edges by dst, buckets per (core, dst-tile), splits by src<32768
  (dma_gather idx is int16) and pads each bucket to 128-slot chunks.
- L1: gather x_bf16[src] rows (256B) from HBM via gpsimd.dma_gather;
  scatter-add via one-hot matmuls into PSUM (one-hot built on DVE with
  iota + is_equal against per-slot dst values); mean via per-partition
  inv-degree scale; dense W1_l/W1_r matmuls (f32) fused bias+relu on ACT.
- h kept transposed [hid, nodes] in SBUF; p = h @ W2_l computed row-major,
  AllGathered (bf16, 128-col padded rows) so every core can gather p[src].
- L2: same gather/scatter machinery on p; + h @ W2_r + b2; log_softmax
  along the free dim; DMA out.
"""

import numpy as np
import ml_dtypes

import concourse.bacc as bacc
import concourse.bass as bass
import concourse.mybir as mybir
import concourse.tile as tile
from concourse.bass_utils import run_bass_kernel_spmd

N = 50000
F = 128
HID = 256
CLS = 47
CORES = 8
NPC = N // CORES           # 6250
TPC = (NPC + 127) // 128   # 49 tiles per core
SPLIT = 32768              # int16 index limit for dma_gather
GPT = 7                    # dst-tiles per gather group
NG = (TPC + GPT - 1) // GPT

f32 = mybir.dt.float32
bf16 = mybir.dt.bfloat16
i16 = mybir.dt.int16
ALU = mybir.AluOpType
ACTF = mybir.ActivationFunctionType

IOTA_BF = np.tile(np.arange(128, dtype=np.float32)[None, :],
                  (128, 1)).astype(ml_dtypes.bfloat16)
IDENT_F32 = np.eye(128, dtype=np.float32)


def _host_prep(x, edge_index):
    src = np.asarray(edge_index[0], np.int64)
    dst = np.asarray(edge_index[1], np.int64)
    deg = np.bincount(dst, minlength=N).astype(np.float32)

    order = np.argsort(dst, kind="stable")
    src_s = src[order]
    dst_s = dst[order]
    bounds = np.searchsorted(dst_s, np.arange(0, N + 1, NPC))

    seg_idx = {}
    cnt = np.zeros((CORES, TPC, 2), np.int64)
    for c in range(CORES):
        sl = slice(bounds[c], bounds[c + 1])
        sc = src_s[sl]
        dcl = dst_s[sl] - c * NPC
        tt = dcl >> 7
        t_ord = np.argsort(tt, kind="stable")
        sc, dcl, tt = sc[t_ord], dcl[t_ord], tt[t_ord]
        tb = np.searchsorted(tt, np.arange(TPC + 1))
        for t in range(TPC):
            s2 = slice(tb[t], tb[t + 1])
            s_t = sc[s2]
            d_t = dcl[s2] & 127
            lo = s_t < SPLIT
            seg_idx[(c, t, 0)] = (s_t[lo], d_t[lo])
            seg_idx[(c, t, 1)] = (s_t[~lo] - SPLIT, d_t[~lo])
            cnt[c, t, 0] = int(lo.sum())
            cnt[c, t, 1] = int((~lo).sum())

    # chunk counts, uniform across cores (SPMD single program)
    nch = np.ceil(cnt / 128.0).astype(np.int64).max(axis=0)  # [TPC, 2]

    groups = []
    chunk_ptr = 0
    for g in range(NG):
        tiles = list(range(g * GPT, min((g + 1) * GPT, TPC)))
        seg_chunks = {0: {}, 1: {}}
        base = chunk_ptr
        for s in (0, 1):
            for t in tiles:
                seg_chunks[s][t] = (chunk_ptr, int(nch[t, s]))
                chunk_ptr += int(nch[t, s])
        groups.append(dict(tiles=tiles, seg_chunks=seg_chunks, base=base,
                           nchunks=chunk_ptr - base))
    tot_ch = chunk_ptr
    W = tot_ch * 8  # idx columns: 128 slots/chunk / 16

    gidx_all, dstv_all, degp_all, xown_all = [], [], [], []
    for c in range(CORES):
        gi = np.zeros((16, W), np.int16)
        dv = np.full((128, tot_ch), -1.0, np.float32)
        for t in range(TPC):
            g = t // GPT
            for s in (0, 1):
                c0, ncks = groups[g]["seg_chunks"][s][t]
                if ncks == 0:
                    continue
                iv, dl = seg_idx[(c, t, s)]
                S = ncks * 128
                ivp = np.zeros(S, np.int64)
                ivp[: len(iv)] = iv
                dvp = np.full(S, -1.0, np.float32)
                dvp[: len(dl)] = dl
                gi[:, c0 * 8:(c0 + ncks) * 8] = ivp.reshape(-1, 16).T
                dv[:, c0:c0 + ncks] = dvp.reshape(ncks, 128).T
        gidx_all.append(np.tile(gi, (8, 1)))  # replicate across 8 Q7 cores
        dstv_all.append(dv)
        dpc = np.ones(TPC * 128, np.float32)
        dpc[:NPC] = deg[c * NPC:(c + 1) * NPC]
        degp_all.append(np.ascontiguousarray(dpc.reshape(TPC, 128).T))
        xo = np.zeros((TPC * 128, F), np.float32)
        xo[:NPC] = x[c * NPC:(c + 1) * NPC]
        xown_all.append(xo)

    sched = dict(groups=groups, tot_ch=tot_ch, W=W,
                 max_gch=max(g["nchunks"] for g in groups))
    return sched, gidx_all, dstv_all, degp_all, xown_all


def _build(sched, phases=3):
    groups, tot_ch, W = sched["groups"], sched["tot_ch"], sched["W"]
    max_gch = sched["max_gch"]

    nc = bacc.Bacc("TRN2", num_devices=CORES)
    xbf_h = nc.declare_dram_parameter("xbf", [N, F], bf16, False)
    xown_h = nc.declare_dram_parameter("xown", [TPC * 128, F], f32, False)
    gidx_h = nc.declare_dram_parameter("gidx", [128, W], i16, False)
    iotab_h = nc.declare_dram_parameter("iotab", [128, 128], bf16, False)
    ident_h = nc.declare_dram_parameter("ident", [128, 128], f32, False)
    dstv_h = nc.declare_dram_parameter("dstv", [128, tot_ch], f32, False)
    degp_h = nc.declare_dram_parameter("degp", [128, TPC], f32, False)
    w1l_h = nc.declare_dram_parameter("w1l", [F, HID], f32, False)
    w1r_h = nc.declare_dram_parameter("w1r", [F, HID], f32, False)
    w2l_h = nc.declare_dram_parameter("w2l", [128, 2 * CLS], f32, False)
    w2r_h = nc.declare_dram_parameter("w2r", [128, 2 * CLS], f32, False)
    b1_h = nc.declare_dram_parameter("b1c", [128, 2], f32, False)
    b2_h = nc.declare_dram_parameter("b2r", [1, CLS], f32, False)
    out_h = nc.declare_dram_parameter("out", [NPC, CLS], f32, True)

    p_loc = nc.dram_tensor("p_loc", [NPC, 128], bf16)
    p_full = nc.dram_tensor("p_full", [N, 128], bf16, addr_space="Shared")

    with tile.TileContext(nc) as tc:
        with (
            tc.tile_pool(name="const", bufs=1) as cp,
            tc.tile_pool(name="msg", bufs=2) as msgp,
            tc.tile_pool(name="oh", bufs=6) as ohp,
            tc.tile_pool(name="sb", bufs=3) as sbp,
            tc.tile_pool(name="small", bufs=4) as smp,
        ):
            # ---- persistent tiles ----
            idx_sb = cp.tile([128, W], i16, tag="idx")
            nc.sync.dma_start(idx_sb[:], gidx_h[:, :])
            dstv_sb = cp.tile([128, tot_ch], f32, tag="dstv")
            nc.sync.dma_start(dstv_sb[:], dstv_h[:, :])
            w1l_sb = cp.tile([F, HID], f32, tag="w1l")
            nc.sync.dma_start(w1l_sb[:], w1l_h[:, :])
            w1r_sb = cp.tile([F, HID], f32, tag="w1r")
            nc.sync.dma_start(w1r_sb[:], w1r_h[:, :])
            w2l_sb = cp.tile([128, 2 * CLS], f32, tag="w2l")
            nc.sync.dma_start(w2l_sb[:], w2l_h[:, :])
            w2r_sb = cp.tile([128, 2 * CLS], f32, tag="w2r")
            nc.sync.dma_start(w2r_sb[:], w2r_h[:, :])
            b1_sb = cp.tile([128, 2], f32, tag="b1")
            nc.sync.dma_start(b1_sb[:], b1_h[:, :])
            b2_sb = cp.tile([1, CLS], f32, tag="b2")
            nc.sync.dma_start(b2_sb[:], b2_h[:, :])
            deg_sb = cp.tile([128, TPC], f32, tag="deg")
            nc.sync.dma_start(deg_sb[:], degp_h[:, :])

            inv_sb = cp.tile([128, TPC], f32, tag="inv")
            nc.vector.tensor_scalar(inv_sb[:], deg_sb[:], 1.0, None, ALU.max)
            nc.vector.reciprocal(inv_sb[:], inv_sb[:])

            iota_bf = cp.tile([128, 128], bf16, tag="iotabf")
            nc.sync.dma_start(iota_bf[:], iotab_h[:, :])
            ident = cp.tile([128, 128], f32, tag="ident")
            nc.sync.dma_start(ident[:], ident_h[:, :])
            ones_sb = cp.tile([1, 128], f32, tag="ones")
            nc.vector.memset(ones_sb[:], 1.0)

            h1T0 = cp.tile([128, TPC * 128], f32, tag="h1a")
            h1T1 = cp.tile([128, TPC * 128], f32, tag="h1b")

            def gathers(group, table_lo, table_hi, msg3):
                """Issue lo/hi dma_gather for one group into msg3 [128,C,128]."""
                base = group["base"]
                n_lo = sum(n for (_, n) in group["seg_chunks"][0].values())
                n_hi = sum(n for (_, n) in group["seg_chunks"][1].values())
                if n_lo:
                    S = n_lo * 128
                    nc.gpsimd.dma_gather(
                        msg3[:, 0:n_lo, :], table_lo,
                        idx_sb[:, base * 8:(base + n_lo) * 8],
                        S, S, F, single_packet=False)
                if n_hi:
                    S = n_hi * 128
                    nc.gpsimd.dma_gather(
                        msg3[:, n_lo:n_lo + n_hi, :], table_hi,
                        idx_sb[:, (base + n_lo) * 8:(base + n_lo + n_hi) * 8],
                        S, S, F, single_packet=False)

            def agg_tile_chunks(group, t, msg3, psl):
                """One-hot matmuls accumulating agg for dst-tile t."""
                base = group["base"]
                lo0, nlo = group["seg_chunks"][0][t]
                hi0, nhi = group["seg_chunks"][1][t]
                gcs = [lo0 + k for k in range(nlo)] + \
                      [hi0 + k for k in range(nhi)]
                for i, gc in enumerate(gcs):
                    oh = ohp.tile([128, 128], bf16, tag="oh")
                    nc.vector.tensor_scalar(oh[:], iota_bf[:],
                                            dstv_sb[:, gc:gc + 1], None,
                                            ALU.is_equal)
                    nc.tensor.matmul(psl, oh[:], msg3[:, gc - base, :],
                                     start=(i == 0), stop=(i == len(gcs) - 1))
                return len(gcs) > 0

            # =============== Layer 1 ===============
            with (
                tc.tile_pool(name="aggps", bufs=3, space="PSUM") as aggpp,
                tc.tile_pool(name="tp", bufs=2, space="PSUM") as tpp,
                tc.tile_pool(name="zp", bufs=2, space="PSUM") as zpp,
            ):
                for g in range(NG):
                    grp = groups[g]
                    gch = grp["nchunks"]
                    msg = msgp.tile([128, max_gch * 128], bf16, tag="msg")
                    msg3 = msg[:].rearrange("p (c e) -> p c e", e=F)
                    gathers(grp, xbf_h[0:SPLIT, :], xbf_h[SPLIT:N, :], msg3)
                    for tl, t in enumerate(grp["tiles"]):
                        agg_ps = aggpp.tile([128, 128], f32, tag="agg")
                        nonempty = agg_tile_chunks(grp, t, msg3, agg_ps[:])
                        mean = sbp.tile([128, 128], f32, tag="mean")
                        if nonempty:
                            nc.vector.tensor_scalar(
                                mean[:], agg_ps[:],
                                inv_sb[:, t:t + 1], None, ALU.mult)
                        else:
                            nc.vector.memset(mean[:], 0.0)
                        mt_ps = tpp.tile([128, 128], f32, tag="tp")
                        nc.tensor.transpose(mt_ps[:], mean[:], ident[:])
                        meanT = sbp.tile([128, 128], f32, tag="meanT")
                        nc.scalar.activation(meanT[:], mt_ps[:], ACTF.Copy)
                        xo = sbp.tile([128, 128], f32, tag="xo")
                        nc.sync.dma_start(xo[:], xown_h[t * 128:(t + 1) * 128, :])
                        xt_ps = tpp.tile([128, 128], f32, tag="tp")
                        nc.tensor.transpose(xt_ps[:], xo[:], ident[:])
                        xoT = sbp.tile([128, 128], f32, tag="xoT")
                        nc.scalar.activation(xoT[:], xt_ps[:], ACTF.Copy)
                        z_ps = zpp.tile([128, 256], f32, tag="z")
                        for h, h1T in ((0, h1T0), (1, h1T1)):
                            zs = z_ps[:, h * 128:(h + 1) * 128]
                            nc.tensor.matmul(zs, w1l_sb[:, h * 128:(h + 1) * 128],
                                             meanT[:], start=True, stop=False)
                            nc.tensor.matmul(zs, w1r_sb[:, h * 128:(h + 1) * 128],
                                             xoT[:], start=False, stop=True)
                            nc.scalar.activation(h1T[:, t * 128:(t + 1) * 128],
                                                 zs, ACTF.Relu,
                                                 bias=b1_sb[:, h:h + 1],
                                                 scale=1.0)

            # =============== p = h @ W2_l, AllGather ===============
            with tc.tile_pool(name="pp", bufs=2, space="PSUM") as ppp:
                if phases < 2:
                    for t in range(TPC):
                        res = smp.tile([128, CLS], f32, tag="res")
                        nc.vector.tensor_copy(res[:], h1T0[:, t * 128:t * 128 + CLS])
                        rows = NPC - t * 128 if t == TPC - 1 else 128
                        nc.sync.dma_start(out_h[t * 128:t * 128 + rows, :], res[0:rows, :])
                for t in (range(TPC) if phases >= 2 else []):
                    ts = slice(t * 128, (t + 1) * 128)
                    pp_ps = ppp.tile([128, 64], f32, tag="pp")
                    nc.tensor.matmul(pp_ps[:, 0:CLS], h1T0[:, ts],
                                     w2l_sb[:, 0:CLS], start=True, stop=False)
                    nc.tensor.matmul(pp_ps[:, 0:CLS], h1T1[:, ts],
                                     w2l_sb[:, CLS:2 * CLS], start=False,
                                     stop=True)
                    psb = sbp.tile([128, 128], bf16, tag="psb")
                    nc.vector.memset(psb[:, CLS:128], 0.0)
                    nc.scalar.activation(psb[:, 0:CLS], pp_ps[:, 0:CLS],
                                         ACTF.Copy)
                    rows = NPC - t * 128 if t == TPC - 1 else 128
                    nc.sync.dma_start(p_loc[t * 128:t * 128 + rows, :],
                                      psb[0:rows, :])

                if phases >= 2:
                    nc.gpsimd.collective_compute(
                        "AllGather", ALU.bypass,
                        replica_groups=[list(range(CORES))],
                        ins=[p_loc.ap().opt()], outs=[p_full.ap().opt()])

                # b2 broadcast across partitions via rank-1 matmul
                b2_ps = ppp.tile([128, 64], f32, tag="pp")
                nc.tensor.matmul(b2_ps[:, 0:CLS], ones_sb[0:1, :],
                                 b2_sb[0:1, :], start=True, stop=True)
                b2bc = cp.tile([128, CLS], f32, tag="b2bc")
                nc.scalar.activation(b2bc[:], b2_ps[:, 0:CLS], ACTF.Copy)

            # =============== Layer 2 ===============
            with (
                tc.tile_pool(name="aggps2", bufs=3, space="PSUM") as aggpp2,
                tc.tile_pool(name="op", bufs=2, space="PSUM") as opp,
            ):
                if phases == 2:
                    for t in range(TPC):
                        res = smp.tile([128, CLS], f32, tag="res")
                        nc.vector.tensor_copy(res[:], h1T0[:, t * 128:t * 128 + CLS])
                        rows = NPC - t * 128 if t == TPC - 1 else 128
                        nc.sync.dma_start(out_h[t * 128:t * 128 + rows, :], res[0:rows, :])
                for g in (range(NG) if phases >= 3 else []):
                    grp = groups[g]
                    msg = msgp.tile([128, max_gch * 128], bf16, tag="msg")
                    msg3 = msg[:].rearrange("p (c e) -> p c e", e=F)
                    gathers(grp, p_full[0:SPLIT, :], p_full[SPLIT:N, :], msg3)
                    for tl, t in enumerate(grp["tiles"]):
                        agg_ps = aggpp2.tile([128, 128], f32, tag="agg2")
                        nonempty = agg_tile_chunks(grp, t, msg3, agg_ps[:])
                        ts = slice(t * 128, (t + 1) * 128)
                        o_ps = opp.tile([128, 64], f32, tag="op")
                        nc.tensor.matmul(o_ps[:, 0:CLS], h1T0[:, ts],
                                         w2r_sb[:, 0:CLS], start=True,
                                         stop=False)
                        nc.tensor.matmul(o_ps[:, 0:CLS], h1T1[:, ts],
                                         w2r_sb[:, CLS:2 * CLS], start=False,
                                         stop=True)
                        s_sb = smp.tile([128, CLS], f32, tag="s")
                        if nonempty:
                            nc.vector.tensor_scalar(
                                s_sb[:],
                                agg_ps[:, 0:CLS],
                                inv_sb[:, t:t + 1], None, ALU.mult)
                        else:
                            nc.vector.memset(s_sb[:], 0.0)
                        lg = smp.tile([128, CLS], f32, tag="lg")
                        nc.vector.tensor_tensor(lg[:], o_ps[:, 0:CLS], s_sb[:],
                                                ALU.add)
                        lg2 = smp.tile([128, CLS], f32, tag="lg2")
                        nc.vector.tensor_tensor(lg2[:], lg[:], b2bc[:], ALU.add)
                        mx = smp.tile([128, 1], f32, tag="mx")
                        nc.vector.tensor_reduce(mx[:], lg2[:],
                                                mybir.AxisListType.X, ALU.max)
                        sh = smp.tile([128, CLS], f32, tag="sh")
                        nc.vector.tensor_scalar(sh[:], lg2[:], mx[:, 0:1], None,
                                                ALU.subtract)
                        ex = smp.tile([128, CLS], f32, tag="ex")
                        nc.scalar.activation(ex[:], sh[:], ACTF.Exp)
                        sm = smp.tile([128, 1], f32, tag="sm")
                        nc.vector.tensor_reduce(sm[:], ex[:],
                                                mybir.AxisListType.X, ALU.add)
                        ls = smp.tile([128, 1], f32, tag="ls")
                        nc.scalar.activation(ls[:], sm[:], ACTF.Ln)
                        res = smp.tile([128, CLS], f32, tag="res")
                        nc.vector.tensor_scalar(res[:], sh[:], ls[:, 0:1], None,
                                                ALU.subtract)
                        rows = NPC - t * 128 if t == TPC - 1 else 128
                        nc.sync.dma_start(out_h[t * 128:t * 128 + rows, :],
                                          res[0:rows, :])

    nc.compile()
    return nc




def _make_in_maps(inputs, gidx_all, dstv_all, degp_all, xown_all):
    x = np.asarray(inputs["x"], np.float32)
    xbf = np.asarray(x, ml_dtypes.bfloat16)
    w2lf = np.asarray(inputs["W2_l"], np.float32)
    w2rf = np.asarray(inputs["W2_r"], np.float32)
    w2l = np.ascontiguousarray(np.concatenate([w2lf[:128, :], w2lf[128:, :]], axis=1))
    w2r = np.ascontiguousarray(np.concatenate([w2rf[:128, :], w2rf[128:, :]], axis=1))
    b1c = np.ascontiguousarray(np.asarray(inputs["b1"], np.float32).reshape(2, 128).T)
    b2r = np.ascontiguousarray(np.asarray(inputs["b2"], np.float32).reshape(1, CLS))
    w1l = np.ascontiguousarray(np.asarray(inputs["W1_l"], np.float32))
    w1r = np.ascontiguousarray(np.asarray(inputs["W1_r"], np.float32))
    in_maps = []
    for c in range(CORES):
        in_maps.append({
            "xbf": xbf,
            "xown": xown_all[c],
            "gidx": gidx_all[c],
            "dstv": dstv_all[c],
            "degp": degp_all[c],
            "w1l": w1l, "w1r": w1r, "w2l": w2l, "w2r": w2r,
            "b1c": b1c, "b2r": b2r,
            "iotab": IOTA_BF, "ident": IDENT_F32,
        })
    return in_maps


def _run(inputs, trace=False):
    x = np.asarray(inputs["x"], np.float32)
    edge_index = np.asarray(inputs["edge_index"])
    sched, gidx_all, dstv_all, degp_all, xown_all = _host_prep(x, edge_index)
    nc = _build(sched)
    in_maps = _make_in_maps(inputs, gidx_all, dstv_all, degp_all, xown_all)
    res = run_bass_kernel_spmd(nc, in_maps, core_ids=list(range(CORES)),
                               trace=trace)
    out = np.concatenate([r["out"] for r in res.results], axis=0)
    return out, res


def kernel(**inputs):
    out, _ = _run(inputs, trace=False)
    return out



# revision 28
# speedup vs baseline: 4922.6592x; 4922.6592x over previous
"""GraphSAGE 2-layer forward on 8 TRN2 NeuronCores — v3.

Design:
- dst nodes sharded naturally (6250/core, padded to D=6272 columns).
- Aggregation: non-transpose dma_gather of 256B feature rows (slots =
  edges sorted by (src-table-half, dst)), spread round-robin over 4 SWDGE
  queues (~2.3x descgen throughput). Scatter-add via FLIPPED one-hot
  matmuls: stationary = gathered 128-slot msg group [slot, feat], moving
  = narrow host-streamed segment matrix oh[slot, dstcol] with 1/deg
  folded into its values -> PSUM window [feat, 512 dstcols] accumulates
  meanT directly (no transposes, no DVE one-hot builds).
- Two passes per layer (lo/hi src table half, int16 gather idx limit);
  hi-pass windows add onto the lo-pass SBUF accumulator via DVE.
- Dense algebra in transposed layouts: h1T = relu(W1l.T@meanT +
  W1r.T@xT + b1) via wide moving operands; p = h1@W2_l row-major per
  tile; AllGather p; L2 same aggregation on p rows; out per tile with
  PE transpose of mean2T + log_softmax.
"""

import numpy as np
import ml_dtypes

import concourse.bacc as bacc
import concourse.bass as bass
import concourse.mybir as mybir
import concourse.tile as tile
from concourse.bass_utils import run_bass_kernel_spmd

N = 50000
F = 128
HID = 256
CLS = 47
CORES = 8
NPC = N // CORES            # 6250
D = 6272                    # padded columns per core (49*128)
CH = 4096                   # slots per gather chunk
MC = 128                    # slots per matmul microchunk
WIN = 512                   # psum window columns
SPLIT = 32768               # int16 gather index limit
D1 = 3072                   # p split: windows 0-5
D2 = D - D1                 # 3200: windows 6-12

f32 = mybir.dt.float32
bf16 = mybir.dt.bfloat16
i16 = mybir.dt.int16
ALU = mybir.AluOpType
ACTF = mybir.ActivationFunctionType

IDENT_F32 = np.eye(128, dtype=np.float32)
NW = (D + WIN - 1) // WIN   # windows (last one narrower)


def _win_width(w):
    return min(WIN, D - w * WIN)


def _layer_struct(idx_slots, col_slots, val_slots, S, S_half):
    """Uniform chunk/microchunk/emission structure + per-core idx/oh.
    S_half: list of per-pass padded slot counts (any number of passes)."""
    offs = np.concatenate([[0], np.cumsum(S_half)])
    chunk_list = []
    for h in range(len(S_half)):
        off = int(offs[h])
        s0 = 0
        while s0 < S_half[h]:
            n = min(CH, S_half[h] - s0)
            chunk_list.append((h, off + s0, n))
            s0 += n
    n_mc = S // MC
    mc_half = np.zeros(n_mc, np.int64)
    for h in range(len(S_half)):
        off = int(offs[h])
        mc_half[off // MC:(off + S_half[h]) // MC] = h
    cmin = np.full(n_mc, 1 << 30, np.int64)
    cmax = np.full(n_mc, -1, np.int64)
    for c in range(CORES):
        cs = col_slots[c].reshape(n_mc, MC)
        valid = cs >= 0
        anyv = valid.any(axis=1)
        lo = np.where(valid, cs, 1 << 30).min(axis=1)
        hi = np.where(valid, cs, -1).max(axis=1)
        cmin = np.minimum(cmin, np.where(anyv, lo, cmin))
        cmax = np.maximum(cmax, np.where(anyv, hi, cmax))

    emissions = []
    oh_off = 0
    win_first, win_last = {}, {}
    eid = 0
    for m in range(n_mc):
        ems = []
        if cmax[m] >= 0:
            h = int(mc_half[m])
            w0, w1 = int(cmin[m]) // WIN, int(cmax[m]) // WIN
            for w in range(w0, w1 + 1):
                ww = _win_width(w)
                key = (h, w)
                if key not in win_first:
                    c0, cw = w * WIN, ww          # opener: full window
                    win_first[key] = eid
                else:
                    c0 = max(int(cmin[m]), w * WIN)
                    cw = min(int(cmax[m]), w * WIN + ww - 1) - c0 + 1
                win_last[key] = eid
                ems.append((w, c0, cw, oh_off))
                oh_off += cw
                eid += 1
        emissions.append(ems)
    OHW = oh_off
    op_set = set(win_first.values())
    cl_set = {v: k for k, v in win_last.items()}
    # role per (pass, window) closer: copy / add / final
    by_win = {}
    for (h, w) in win_first:
        by_win.setdefault(w, []).append(h)
    role = {}
    for w, hs in by_win.items():
        hs.sort()
        for i, h in enumerate(hs):
            role[(h, w)] = ("copy" if i == 0 else
                            ("final" if i == len(hs) - 1 else "add"))
            if len(hs) == 1:
                role[(h, w)] = "copy_final"
    sched_mcs = []
    eid = 0
    for m in range(n_mc):
        lst = []
        for (w, c0, cw, off) in emissions[m]:
            cl = cl_set.get(eid)
            lst.append(dict(win=w, c0=c0, cw=cw, off=off,
                            opener=(eid in op_set), closer=cl,
                            role=role.get(cl) if cl else None))
            eid += 1
        sched_mcs.append(lst)

    idx_all, oh_all = [], []
    for c in range(CORES):
        oh = np.zeros((128, OHW), np.float32)
        cs = col_slots[c].reshape(n_mc, MC)
        vs = val_slots[c].reshape(n_mc, MC)
        for m in range(n_mc):
            for e in sched_mcs[m]:
                rel = cs[m] - e["c0"]
                ok = (cs[m] >= 0) & (rel >= 0) & (rel < e["cw"])
                p_idx = np.arange(MC)[ok]
                oh[p_idx, e["off"] + rel[ok]] = vs[m][ok]
        oh_all.append(np.ascontiguousarray(oh.astype(ml_dtypes.bfloat16)))
        a = idx_slots[c]
        wrp = np.zeros((128, S // 16), np.int16)
        b = a.reshape(S // 16, 16).T.astype(np.int16)
        for g in range(8):
            wrp[16 * g:16 * (g + 1), :] = b
        idx_all.append(wrp)

    return dict(S=S, S_half=S_half, OHW=OHW, chunks=chunk_list,
                mcs=sched_mcs, n_mc=n_mc, mc_half=mc_half), idx_all, oh_all


def host_prep_all(x, edge_index):
    src = np.asarray(edge_index[0], np.int64)
    dst = np.asarray(edge_index[1], np.int64)
    deg = np.bincount(dst, minlength=N).astype(np.int64)
    invdeg = (1.0 / np.maximum(deg, 1)).astype(np.float32)
    core = dst // NPC

    def build_slots_multi(tab_idx, passid, npass):
        per = {}
        S_half = [0] * npass
        for c in range(CORES):
            for h in range(npass):
                m = (core == c) & (passid == h)
                ti = tab_idx[m]
                d = dst[m]
                o = np.argsort(d, kind="stable")
                per[(c, h)] = (ti[o], d[o] - c * NPC, invdeg[d[o]])
        for h in range(npass):
            mx = max(len(per[(c, h)][0]) for c in range(CORES))
            S_half[h] = ((mx + MC - 1) // MC) * MC
        S = int(sum(S_half))
        offs = np.concatenate([[0], np.cumsum(S_half)]).astype(np.int64)
        idx_slots = np.zeros((CORES, S), np.int64)
        col_slots = np.full((CORES, S), -1, np.int64)
        val_slots = np.zeros((CORES, S), np.float32)
        for c in range(CORES):
            for h in range(npass):
                ti, dl, iv = per[(c, h)]
                off = int(offs[h])
                n = len(ti)
                idx_slots[c, off:off + n] = ti
                col_slots[c, off:off + n] = dl
                val_slots[c, off:off + n] = iv
        return idx_slots, col_slots, val_slots, S, S_half

    def build_slots(tab_idx, halves):
        S_half = [0, 0]
        per = {}
        for c in range(CORES):
            for h in (0, 1):
                m = (core == c) & (halves == h)
                ti = tab_idx[m]
                d = dst[m]
                o = np.argsort(d, kind="stable")
                per[(c, h)] = (ti[o], d[o] - c * NPC, invdeg[d[o]])
        for h in (0, 1):
            mx = max(len(per[(c, h)][0]) for c in range(CORES))
            S_half[h] = ((mx + MC - 1) // MC) * MC
        S = S_half[0] + S_half[1]
        idx_slots = np.zeros((CORES, S), np.int64)
        col_slots = np.full((CORES, S), -1, np.int64)
        val_slots = np.zeros((CORES, S), np.float32)
        for c in range(CORES):
            for h, off in ((0, 0), (1, S_half[0])):
                ti, dl, iv = per[(c, h)]
                n = len(ti)
                idx_slots[c, off:off + n] = ti - h * SPLIT
                col_slots[c, off:off + n] = dl
                val_slots[c, off:off + n] = iv
        return idx_slots, col_slots, val_slots, S, S_half

    i1, c1, v1, S1, S1h = build_slots(src, (src >= SPLIT).astype(np.int64))
    l1, idx1_all, oh1_all = _layer_struct(i1, c1, v1, S1, S1h)

    # L2: 4 passes (self1, self2, other-half1, other-half2)
    owner = src // NPC
    row = src % NPC
    half2 = (row >= D1).astype(np.int64)
    selfmask = (owner == core)
    passid = np.where(selfmask, half2, 2 + half2)
    tabidx = np.where(
        passid == 0, row,
        np.where(passid == 1, row - D1,
                 np.where(passid == 2, owner * D1 + row,
                          owner * D2 + row - D1)))
    i2, c2, v2, S2, S2h = build_slots_multi(tabidx, passid, 4)
    l2, idx2_all, oh2_all = _layer_struct(i2, c2, v2, S2, S2h)
    return l1, l2, idx1_all, oh1_all, idx2_all, oh2_all


def _max_chunk_ohw(lx):
    best = 0
    for (h, s0, nsl) in lx["chunks"]:
        ems = [e for m in range(s0 // MC, (s0 + nsl) // MC)
               for e in lx["mcs"][m]]
        if ems:
            best = max(best, ems[-1]["off"] + ems[-1]["cw"] - ems[0]["off"])
    return best


def _build(l1, l2):
    OHT_W = ((max(_max_chunk_ohw(l1), _max_chunk_ohw(l2)) + 255) // 256) * 256
    nc = bacc.Bacc("TRN2", num_devices=CORES, num_swdge_queues=4)
    xbf_h = nc.declare_dram_parameter("xbf", [N, F], bf16, False)
    idx1_h = nc.declare_dram_parameter("idx1", [128, l1["S"] // 16], i16, False)
    idx2_h = nc.declare_dram_parameter("idx2", [128, l2["S"] // 16], i16, False)
    oh1_h = nc.declare_dram_parameter("oh1", [128, l1["OHW"]], bf16, False)
    oh2_h = nc.declare_dram_parameter("oh2", [128, l2["OHW"]], bf16, False)
    xtc_h = nc.declare_dram_parameter("xtc", [128, D], bf16, False)
    w1l_h = nc.declare_dram_parameter("w1l", [F, HID], bf16, False)
    w1r_h = nc.declare_dram_parameter("w1r", [F, HID], bf16, False)
    w2l_h = nc.declare_dram_parameter("w2l", [128, 2 * CLS], bf16, False)
    w2r_h = nc.declare_dram_parameter("w2r", [128, 2 * CLS], bf16, False)
    b1_h = nc.declare_dram_parameter("b1c", [128, 2], f32, False)
    b2_h = nc.declare_dram_parameter("b2r", [1, CLS], f32, False)
    ident_h = nc.declare_dram_parameter("ident", [128, 128], f32, False)
    out_h = nc.declare_dram_parameter("out", [D, CLS], bf16, True)

    p_loc1 = nc.dram_tensor("p_loc1", [D1, 128], bf16)
    p_loc2 = nc.dram_tensor("p_loc2", [D2, 128], bf16)
    p_full1 = nc.dram_tensor("p_full1", [CORES * D1, 128], bf16,
                             addr_space="Shared")
    p_full2 = nc.dram_tensor("p_full2", [CORES * D2, 128], bf16,
                             addr_space="Shared")

    j_chunks = [(j * WIN, _win_width(j)) for j in range(NW)]

    with tile.TileContext(nc) as tc:
        with (
            tc.tile_pool(name="const", bufs=1) as cp,
            tc.tile_pool(name="msg", bufs=3) as msgp,
            tc.tile_pool(name="oh", bufs=3) as ohp,
            tc.tile_pool(name="sm", bufs=4) as smp,
        ):
            idx1_sb = cp.tile([128, l1["S"] // 16], i16, tag="idx1")
            nc.sync.dma_start(idx1_sb[:], idx1_h[:, :])
            idx2_sb = cp.tile([128, l2["S"] // 16], i16, tag="idx2")
            w1l_sb = cp.tile([F, HID], bf16, tag="w1l")
            nc.sync.dma_start(w1l_sb[:], w1l_h[:, :])
            w1r_sb = cp.tile([F, HID], bf16, tag="w1r")
            nc.sync.dma_start(w1r_sb[:], w1r_h[:, :])
            w2l_sb = cp.tile([128, 2 * CLS], bf16, tag="w2l")
            nc.sync.dma_start(w2l_sb[:], w2l_h[:, :])
            w2r_sb = cp.tile([128, 2 * CLS], bf16, tag="w2r")
            nc.sync.dma_start(w2r_sb[:], w2r_h[:, :])
            b1_sb = cp.tile([128, 2], f32, tag="b1")
            nc.sync.dma_start(b1_sb[:], b1_h[:, :])
            b2_sb = cp.tile([1, CLS], f32, tag="b2")
            nc.sync.dma_start(b2_sb[:], b2_h[:, :])
            ident = cp.tile([128, 128], f32, tag="ident")
            nc.sync.dma_start(ident[:], ident_h[:, :])
            ones_sb = cp.tile([1, 128], f32, tag="ones")
            nc.vector.memset(ones_sb[:], 1.0)
            identb = cp.tile([128, 128], bf16, tag="identb")
            nc.vector.tensor_copy(identb[:], ident[:])

            h1T = cp.tile([128, 2, D], bf16, tag="h1T")

            def agg_layer(lx, idx_sb, oh_h, tables, meanT, mean_lo, winp,
                          on_hi_close=None):
                chunks, mcs = lx["chunks"], lx["mcs"]
                win_tiles = {}
                for ci, (h, s0, nsl) in enumerate(chunks):
                    msg = msgp.tile([128, CH // 128, 128], bf16, tag="msg")
                    nc.gpsimd.dma_gather(
                        msg[:, 0:nsl // 128, :], tables[h],
                        idx_sb[:, s0 // 16:(s0 + nsl) // 16],
                        nsl, nsl, F, single_packet=False, queue_num=ci % 4)
                    ems = [e for m in range(s0 // MC, (s0 + nsl) // MC)
                           for e in mcs[m]]
                    if not ems:
                        continue
                    o0 = ems[0]["off"]
                    o1 = ems[-1]["off"] + ems[-1]["cw"]
                    ohw = o1 - o0
                    assert ohw <= OHT_W, ohw
                    oht = ohp.tile([128, OHT_W], bf16, tag="oh")
                    nc.sync.dma_start(oht[:, 0:ohw], oh_h[:, o0:o1])
                    for mi, m in enumerate(range(s0 // MC, (s0 + nsl) // MC)):
                        for e in mcs[m]:
                            w = e["win"]
                            key = (h, w)
                            if e["opener"]:
                                win_tiles[key] = winp.tile(
                                    [128, WIN], f32, tag="win",
                                    name=f"win_{h}_{w}")
                            ps = win_tiles[key]
                            rel = e["c0"] - w * WIN
                            nc.tensor.matmul(
                                ps[:, rel:rel + e["cw"]],
                                msg[:, mi, :],
                                oht[:, e["off"] - o0:e["off"] - o0 + e["cw"]],
                                start=e["opener"],
                                stop=(e["closer"] is not None))
                            if e["closer"] is not None:
                                hh, ww = e["closer"]
                                wid = _win_width(ww)
                                cws = slice(ww * WIN, ww * WIN + wid)
                                role = e["role"]
                                if role == "copy":
                                    nc.scalar.activation(
                                        mean_lo[:, cws], ps[:, 0:wid],
                                        ACTF.Copy)
                                elif role == "add":
                                    nc.vector.tensor_tensor(
                                        mean_lo[:, cws], ps[:, 0:wid],
                                        mean_lo[:, cws], ALU.add)
                                elif role == "copy_final":
                                    nc.scalar.activation(
                                        meanT[:, cws], ps[:, 0:wid],
                                        ACTF.Copy)
                                    if on_hi_close is not None:
                                        on_hi_close(ww)
                                else:  # final
                                    nc.vector.tensor_tensor(
                                        meanT[:, cws], ps[:, 0:wid],
                                        mean_lo[:, cws], ALU.add)
                                    if on_hi_close is not None:
                                        on_hi_close(ww)
                                del win_tiles[key]

            # =============== Layer 1 ===============
            with (
                tc.tile_pool(name="l1", bufs=1) as l1p,
                tc.tile_pool(name="pp", bufs=2, space="PSUM") as ppp,
            ):
                mean_lo = l1p.tile([128, D], bf16, tag="mean_lo")
                meanT = l1p.tile([128, D], bf16, tag="meanT")
                xtc_sb = l1p.tile([128, D], bf16, tag="xtc")
                nc.sync.dma_start(xtc_sb[:], xtc_h[:, :])

                closed_w = set()
                cc_done = [False, False]

                def l1_close(w):
                    j0, jn = w * WIN, _win_width(w)
                    for hh in (0, 1):
                        z = ppp.tile([128, WIN], f32, tag="z",
                                     name=f"z_{w}_{hh}")
                        nc.tensor.matmul(
                            z[:, 0:jn],
                            w1l_sb[:, hh * 128:(hh + 1) * 128],
                            meanT[:, j0:j0 + jn], start=True, stop=False)
                        nc.tensor.matmul(
                            z[:, 0:jn],
                            w1r_sb[:, hh * 128:(hh + 1) * 128],
                            xtc_sb[:, j0:j0 + jn], start=False, stop=True)
                        nc.scalar.activation(
                            h1T[:, hh, j0:j0 + jn], z[:, 0:jn], ACTF.Relu,
                            bias=b1_sb[:, hh:hh + 1], scale=1.0)
                    for t in range(j0 // 128, (j0 + jn) // 128):
                        ts = slice(t * 128, (t + 1) * 128)
                        pp_ps = ppp.tile([128, 64], f32, tag="pp",
                                         name=f"pp_{t}")
                        nc.tensor.matmul(pp_ps[:, 0:CLS], h1T[:, 0, ts],
                                         w2l_sb[:, 0:CLS], start=True,
                                         stop=False)
                        nc.tensor.matmul(pp_ps[:, 0:CLS], h1T[:, 1, ts],
                                         w2l_sb[:, CLS:2 * CLS], start=False,
                                         stop=True)
                        psb = smp.tile([128, 128], bf16, tag="psb",
                                       name=f"psb_{t}")
                        nc.vector.memset(psb[:, CLS:128], 0.0)
                        nc.scalar.activation(psb[:, 0:CLS], pp_ps[:, 0:CLS],
                                             ACTF.Copy)
                        r0 = t * 128
                        if r0 < D1:
                            nc.sync.dma_start(p_loc1[r0:r0 + 128, :], psb[:])
                        else:
                            nc.sync.dma_start(
                                p_loc2[r0 - D1:r0 - D1 + 128, :], psb[:])
                    closed_w.add(w)
                    if closed_w >= set(range(D1 // WIN)) and not cc_done[0]:
                        nc.gpsimd.collective_compute(
                            "AllGather", ALU.bypass,
                            replica_groups=[list(range(CORES))],
                            ins=[p_loc1.ap().opt()],
                            outs=[p_full1.ap().opt()])
                        cc_done[0] = True
                    if closed_w >= set(range(NW)) and not cc_done[1]:
                        nc.gpsimd.collective_compute(
                            "AllGather", ALU.bypass,
                            replica_groups=[list(range(CORES))],
                            ins=[p_loc2.ap().opt()],
                            outs=[p_full2.ap().opt()])
                        cc_done[1] = True

                nc.sync.dma_start(idx2_sb[:], idx2_h[:, :])
                with tc.tile_pool(name="win", bufs=3, space="PSUM") as winp:
                    agg_layer(l1, idx1_sb, oh1_h,
                              (xbf_h[0:SPLIT, :], xbf_h[SPLIT:N, :]),
                              meanT, mean_lo, winp, on_hi_close=l1_close)

                b2_ps = ppp.tile([128, 64], f32, tag="pp")
                nc.tensor.matmul(b2_ps[:, 0:CLS], ones_sb[0:1, :],
                                 b2_sb[0:1, :], start=True, stop=True)
                b2bc = cp.tile([128, CLS], f32, tag="b2bc")
                nc.scalar.activation(b2bc[:], b2_ps[:, 0:CLS], ACTF.Copy)

                # L2 dense term h1 @ W2_r precomputed (overlaps AllGather)
                o2_all = cp.tile([128, (D // 128) * CLS], bf16, tag="o2a")
                for t in range(D // 128):
                    ts = slice(t * 128, (t + 1) * 128)
                    o_ps = ppp.tile([128, 64], f32, tag="pp",
                                    name=f"ops_{t}")
                    nc.tensor.matmul(o_ps[:, 0:CLS], h1T[:, 0, ts],
                                     w2r_sb[:, 0:CLS], start=True, stop=False)
                    nc.tensor.matmul(o_ps[:, 0:CLS], h1T[:, 1, ts],
                                     w2r_sb[:, CLS:2 * CLS], start=False,
                                     stop=True)
                    nc.scalar.activation(o2_all[:, t * CLS:(t + 1) * CLS],
                                         o_ps[:, 0:CLS], ACTF.Copy)

            # =============== Layer 2 ===============
            with (
                tc.tile_pool(name="l2", bufs=1) as l2p,
                tc.tile_pool(name="tp", bufs=2, space="PSUM") as tpp,
            ):
                mean2_lo = l2p.tile([128, D], bf16, tag="mean2_lo")
                mean2 = l2p.tile([128, D], bf16, tag="mean2")
                DT = D // 128
                lg2_all = l2p.tile([128, DT * CLS], bf16, tag="lg2a")
                ex_all = l2p.tile([128, DT * CLS], bf16, tag="exa")
                sm_all = l2p.tile([128, DT], f32, tag="sma")

                def l2_close(w):
                    j0, jn = w * WIN, _win_width(w)
                    for t in range(j0 // 128, (j0 + jn) // 128):
                        ts = slice(t * 128, (t + 1) * 128)
                        cs = slice(t * CLS, (t + 1) * CLS)
                        m2t = tpp.tile([128, 64], bf16, tag="tp",
                                       name=f"m2t_{t}")
                        nc.tensor.transpose(m2t[:, 0:CLS],
                                            mean2[0:CLS, ts],
                                            identb[0:CLS, 0:CLS])
                        m2s = smp.tile([128, CLS], f32, tag="m2s",
                                       name=f"m2s_{t}")
                        nc.vector.tensor_copy(m2s[:], m2t[:, 0:CLS])
                        lg = smp.tile([128, CLS], f32, tag="lg",
                                      name=f"lg_{t}")
                        nc.vector.tensor_tensor(lg[:], o2_all[:, cs],
                                                m2s[:], ALU.add)
                        nc.vector.tensor_tensor(lg2_all[:, cs], lg[:],
                                                b2bc[:], ALU.add)
                        nc.scalar.activation(ex_all[:, cs], lg2_all[:, cs],
                                             ACTF.Exp)
                        nc.vector.tensor_reduce(sm_all[:, t:t + 1],
                                                ex_all[:, cs],
                                                mybir.AxisListType.X, ALU.add)

                with tc.tile_pool(name="win2", bufs=3, space="PSUM") as winp2:
                    agg_layer(l2, idx2_sb, oh2_h,
                              (p_loc1.ap()[:, :], p_loc2.ap()[:, :],
                               p_full1.ap()[:, :], p_full2.ap()[:, :]),
                              mean2, mean2_lo, winp2, on_hi_close=l2_close)
                ls_all = l2p.tile([128, DT], f32, tag="lsa")
                nc.scalar.activation(ls_all[:], sm_all[:], ACTF.Ln)
                res_all = l2p.tile([128, DT * CLS], bf16, tag="resa")
                for t in range(DT):
                    cs = slice(t * CLS, (t + 1) * CLS)
                    nc.vector.tensor_scalar(res_all[:, cs], lg2_all[:, cs],
                                            ls_all[:, t:t + 1], None,
                                            ALU.subtract)
                out_v = out_h.ap().rearrange("(t p) c -> p t c", p=128)
                nc.sync.dma_start(
                    out_v, res_all[:].rearrange("p (t c) -> p t c", c=CLS))

    nc.compile()
    return nc


def _make_in_maps(inputs, l1, l2, idx1_all, oh1_all, idx2_all, oh2_all):
    x = np.asarray(inputs["x"], np.float32)
    xbf = np.asarray(x, ml_dtypes.bfloat16)
    w1l = np.ascontiguousarray(np.asarray(inputs["W1_l"], np.float32)
                               .astype(ml_dtypes.bfloat16))
    w1r = np.ascontiguousarray(np.asarray(inputs["W1_r"], np.float32)
                               .astype(ml_dtypes.bfloat16))
    w2lf = np.asarray(inputs["W2_l"], np.float32)
    w2rf = np.asarray(inputs["W2_r"], np.float32)
    w2l = np.ascontiguousarray(
        np.concatenate([w2lf[:128, :], w2lf[128:, :]], axis=1)
        .astype(ml_dtypes.bfloat16))
    w2r = np.ascontiguousarray(
        np.concatenate([w2rf[:128, :], w2rf[128:, :]], axis=1)
        .astype(ml_dtypes.bfloat16))
    b1c = np.ascontiguousarray(
        np.asarray(inputs["b1"], np.float32).reshape(2, 128).T)
    b2r = np.ascontiguousarray(
        np.asarray(inputs["b2"], np.float32).reshape(1, CLS))

    in_maps = []
    for c in range(CORES):
        xt = np.zeros((128, D), np.float32)
        xt[:, 0:NPC] = x[c * NPC:(c + 1) * NPC].T
        in_maps.append({
            "xbf": xbf,
            "idx1": idx1_all[c], "idx2": idx2_all[c],
            "oh1": oh1_all[c], "oh2": oh2_all[c],
            "xtc": np.ascontiguousarray(xt.astype(ml_dtypes.bfloat16)),
            "w1l": w1l, "w1r": w1r, "w2l": w2l, "w2r": w2r,
            "b1c": b1c, "b2r": b2r, "ident": IDENT_F32,
        })
    return in_maps


def _run(inputs, trace=False, tmpdir=None):
    x = np.asarray(inputs["x"], np.float32)
    edge_index = np.asarray(inputs["edge_index"])
    l1, l2, idx1_all, oh1_all, idx2_all, oh2_all = host_prep_all(x, edge_index)
    nc = _build(l1, l2)
    in_maps = _make_in_maps(inputs, l1, l2, idx1_all, oh1_all, idx2_all,
                            oh2_all)
    res = run_bass_kernel_spmd(nc, in_maps, core_ids=list(range(CORES)),
                               trace=trace, tmpdir=tmpdir)
    out = np.concatenate(
        [np.asarray(r["out"][0:NPC], dtype=np.float32) for r in res.results],
        axis=0)
    return out, res


def kernel(**inputs):
    out, _ = _run(inputs, trace=False)
    return out
